# revision 1
# baseline (speedup 1.0000x reference)
"""AttnBlock (GroupNorm -> q/k/v 1x1 conv -> single-head attention -> proj -> residual)
on 8 Trainium2 NeuronCores.

Sharding: pure data-parallel over batch. x is [B=8, C=512, N=2048]; core b runs the
full attention block on x[b]. No collectives.

Per-core dataflow — all five matmul stages run as plain fp8e4m3 matmuls.  The
PE's moving-operand streaming is byte-rate limited (~74.6ns per [128,512] fp8
moving tile vs 206ns bf16 vs ~350ns f32r, measured), so 1-byte operands give
~2.8x the bf16 matmul rate; DoubleRow mode measures SLOWER than plain fp8 on
this hardware and is not used.
  - Weights are host-preprocessed: transposed, scaled by 16 (lifts N(0, 1/C)
    weights out of the fp8 subnormal range), quantized to fp8, as four
    [128, 512] channel-tile slices per weight.  The output projection is
    folded into the v-projection (wv' = wp@wv, exact since softmax rows sum
    to 1; bv and bp fold into bp_eff = bp + wp@bv), so PV directly produces
    the final attention output.  The x16 factors cancel exactly: scores fold
    1/256 into the exp scale, PV folds 1/16 into the softmax-normalizer
    reciprocal (16-valued ones reduce).
  - GroupNorm(32 groups): per-row bn_stats/bn_aggr on DVE, cross-partition
    group reduce/broadcast via tiny f32r PE matmuls with 0/1 selector
    matrices, rstd via a Quake-style rsqrt (bit trick + 2 Newton steps) so
    the Act engine never leaves the exp activation table.  GN for rep r+1 is
    emitted as thunks interleaved into rep r's attention stream so the
    in-order engine queues overlap it with PE work.
  - Attention is software-pipelined over 512-query blocks: score chunks of
    block k interleave with PV (= final output) of block k-1, hiding the
    softmax-normalizer reduce latency.  exp runs on
    Act (fp8 out = PV moving operand), the column-sum tree on Pool, the
    normalizer cross-partition reduce + broadcast on the PE.
  - Engine split for PSUM drains (Pool cannot read PSUM on TRN2): q/k/v
    copies alternate Act (Identity + bias, or Copy) / DVE; the normalizer
    multiply and bias+residual run on DVE; out DMA issues on Pool SWDGE.
"""

import sys

sys.path.insert(0, "/opt/trn_rl_repo")

from contextlib import ExitStack

import numpy as np
import ml_dtypes

import concourse.bass as bass
import concourse.bacc as bacc
import concourse.tile as tile
from concourse import mybir
from concourse.bass_utils import run_bass_kernel_spmd

P = 128
C = 512
N = 2048
B = 8
GROUPS = 32
GSZ = 16  # channels (partition rows) per group
GPT = P // GSZ  # groups per 128-channel tile = 8
CT = C // P  # 4 channel tiles
JC = CT // 2  # 2 channel DoubleRow pairs
NBLK = N // 512  # 4 query blocks of 512
MT = N // P  # 16 key tiles of 128
JM = MT // 2  # 8 key DoubleRow pairs
EPS = 1e-6
# The k-projection is folded into q: u = (wk^T wq) h + wk^T bq, so scores are
# h8 (x1) . u8 (x16) = 16 * S_true; exp applies C^-0.5 / 16 and shifts logits
# by -2 to center the fp8 exp range.  (bk-side bias terms are constant per
# query and cancel in the softmax; the bq-side term rides u's bias exactly.)
SCALE_EFF = float(C) ** -0.5 / 16.0
EXPC = -2.0
# The v-projection weight is wp@wv (host-folded), so PV directly produces the
# final attention output: ps_pv = 16 * csum * out_attn.  A 16-valued ones
# reduce gives Rsb = 1/(16*csum), so ps_pv*Rsb is exact.
ONES_VAL = 16.0
RSQRT_MAGIC = 0x5F3759DF

f32 = mybir.dt.float32
f32r = mybir.dt.float32r
i32 = mybir.dt.int32
f8 = mybir.dt.float8e4
E4 = ml_dtypes.float8_e4m3
OP = mybir.AluOpType
AF = mybir.ActivationFunctionType


def _r(ap):
    return ap.bitcast(f32r)


def build(reps=1):
    nc = bacc.Bacc()

    x_d = nc.declare_dram_parameter("x", [C, N], f32, False)
    gns_d = nc.declare_dram_parameter("gn_scale", [C], f32, False)
    gnb_d = nc.declare_dram_parameter("gn_bias", [C], f32, False)
    w8_d = {}
    for nm in ("wq", "wv"):
        for ci in range(CT):
            w8_d[(nm, ci)] = nc.declare_dram_parameter(f"{nm}8_{ci}", [P, C], f8, False)
    bq16_d = nc.declare_dram_parameter("bq16", [C], f32, False)
    bp_d = nc.declare_dram_parameter("bp_eff", [C], f32, False)
    sel_d = nc.declare_dram_parameter("sel", [P, GPT], f32, False)
    selT_d = nc.declare_dram_parameter("selT", [GPT, P], f32, False)
    out_d = nc.declare_dram_parameter("out", [C, N], f32, True)

    with ExitStack() as ctx:
        tc = ctx.enter_context(tile.TileContext(nc))

        const = ctx.enter_context(tc.tile_pool(name="const", bufs=1))

        # x tiles first: they are the critical path for rep 0.
        xg_pool = ctx.enter_context(tc.tile_pool(name="xg", bufs=1))
        xg = []
        for ci in range(CT):
            xt = xg_pool.tile([P, N], f32, tag=f"xg{ci}", name=f"xg{ci}")
            for hf in range(2):
                nc.sync.dma_start(
                    out=xt[:, hf * 1024 : (hf + 1) * 1024],
                    in_=x_d[ci * P : (ci + 1) * P, hf * 1024 : (hf + 1) * 1024],
                )
            xg.append(xt)

        # weights (already transposed/scaled/fp8 on host) + consts on SWDGE
        w8 = {}
        for (nm, ci), d in w8_d.items():
            t = const.tile([P, C], f8, tag=f"w8_{nm}{ci}", name=f"w8_{nm}{ci}")
            nc.gpsimd.dma_start(out=t, in_=d[:, :])
            w8[(nm, ci)] = t

        sel_sb = const.tile([P, GPT], f32, tag="sel")
        nc.gpsimd.dma_start(out=sel_sb, in_=sel_d[:, :])
        selT_sb = const.tile([GPT, P], f32, tag="selT")
        nc.gpsimd.dma_start(out=selT_sb, in_=selT_d[:, :])

        gs_sb = const.tile([P, CT], f32, tag="gs")
        gb_sb = const.tile([P, CT], f32, tag="gb")
        bq16_sb = const.tile([P, CT], f32, tag="bq16")
        bp_sb = const.tile([P, CT], f32, tag="bp")
        for dst, src_d in (
            (gs_sb, gns_d),
            (gb_sb, gnb_d),
            (bq16_sb, bq16_d),
            (bp_sb, bp_d),
        ):
            nc.gpsimd.dma_start(out=dst, in_=src_d[:].rearrange("(c p) -> p c", p=P))

        ones_col_f = const.tile([P, 1], f32, tag="ones_col_f")
        nc.vector.memset(ones_col_f, ONES_VAL)
        ones_row_f = const.tile([1, P], f32, tag="ones_row_f")
        nc.vector.memset(ones_row_f, 1.0)
        ones_col = const.tile([P, 1], f32r, tag="ones_col")
        ones_row = const.tile([1, P], f32r, tag="ones_row")
        sel_r = const.tile([P, GPT], f32r, tag="sel_r")
        selT_r = const.tile([GPT, P], f32r, tag="selT_r")
        with nc.allow_low_precision(reason="f32r copies of 0/1 selectors"):
            nc.vector.tensor_copy(out=sel_r, in_=sel_sb)
            nc.vector.tensor_copy(out=selT_r, in_=selT_sb)
            nc.vector.tensor_copy(out=ones_col, in_=ones_col_f)
            nc.vector.tensor_copy(out=ones_row, in_=ones_row_f)
        magic_sb = const.tile([GPT, CT], i32, tag="magic")
        nc.vector.memset(magic_sb, RSQRT_MAGIC)
        expc_sb = const.tile([P, 1], f32, tag="expc")
        nc.vector.memset(expc_sb, EXPC)

        env = {
            "xg": xg,
            "w8": w8,
            "const": const,
            "sel_r": sel_r,
            "selT_r": selT_r,
            "gs_sb": gs_sb,
            "gb_sb": gb_sb,
            "bq16_sb": bq16_sb,
            "bp_sb": bp_sb,
            "ones_col": ones_col,
            "ones_row": ones_row,
            "magic_sb": magic_sb,
            "expc_sb": expc_sb,
            "out_d": out_d,
        }
        # persistent pools (tags rotate across reps)
        env["gn_tmp"] = ctx.enter_context(tc.tile_pool(name="gn_tmp", bufs=4))
        env["psG"] = ctx.enter_context(tc.tile_pool(name="psG", bufs=1, space="PSUM"))
        env["psA"] = ctx.enter_context(tc.tile_pool(name="psA", bufs=4, space="PSUM"))
        env["psB"] = ctx.enter_context(tc.tile_pool(name="psB", bufs=2, space="PSUM"))
        env["psR"] = ctx.enter_context(tc.tile_pool(name="psR", bufs=1, space="PSUM"))
        env["pt_pool"] = ctx.enter_context(tc.tile_pool(name="pt", bufs=2))
        env["sm_pool"] = ctx.enter_context(tc.tile_pool(name="sm", bufs=2))
        env["h2_pool"] = ctx.enter_context(tc.tile_pool(name="h2", bufs=2))
        env["out_pool"] = ctx.enter_context(tc.tile_pool(name="outp", bufs=4))

        # GroupNorm for rep 0 runs cold; GN for rep r+1 is emitted as thunks
        # interleaved into rep r's attention (in-order engine queues overlap it
        # with PE work of the previous rep).
        for t in _gn_thunks(nc, tc, env, 0):
            t()
        for rep in range(reps):
            thunks = _gn_thunks(nc, tc, env, rep + 1) if rep + 1 < reps else []
            _build_body(nc, tc, env, rep, thunks)

    nc.finalize()
    return nc


def _gn_thunks(nc, tc, env, rep):
    """GroupNorm for `rep` as a list of closures emitting a few instructions
    each, in dependency order.  Writes h8 pair planes into env['h8_{par}']."""
    par = rep % 2
    xg = env["xg"]
    const = env["const"]
    gn_tmp = env["gn_tmp"]
    psG = env["psG"]
    gs_sb = env["gs_sb"]
    gb_sb = env["gb_sb"]
    magic_sb = env["magic_sb"]
    sel_r = env["sel_r"]
    selT_r = env["selT_r"]

    stats4 = const.tile([P, 2 * CT], f32r, tag=f"stats4_{par}", name="stats4")
    g2 = const.tile([GPT, 2 * CT], f32r, tag=f"g2_{par}", name="g2")
    AB = const.tile([P, 2 * CT], f32, tag=f"AB_{par}", name="AB")
    h8 = [
        const.tile([P, N], f8, tag=f"h8_{par}_{ci}", name=f"h8_{ci}")
        for ci in range(CT)
    ]
    env[f"h8_{par}"] = h8

    thunks = []

    def stats_ci(ci):
        def t():
            st = gn_tmp.tile([P, 4, 6], f32, tag="st", name="st")
            for q4 in range(4):
                nc.vector.bn_stats(
                    out=st[:, q4, :], in_=xg[ci][:, q4 * 512 : (q4 + 1) * 512]
                )
            mv = gn_tmp.tile([P, 2], f32, tag="mv", name="mv")
            nc.vector.bn_aggr(out=mv, in_=st)
            with nc.allow_low_precision(reason="f32r GN stats for PE reduce"):
                nc.vector.tensor_copy(out=stats4[:, ci : ci + 1], in_=mv[:, 0:1])
                nc.vector.tensor_tensor(
                    out=stats4[:, CT + ci : CT + ci + 1],
                    in0=mv[:, 0:1],
                    in1=mv[:, 0:1],
                    op=OP.mult,
                )
                nc.vector.tensor_add(
                    out=stats4[:, CT + ci : CT + ci + 1],
                    in0=stats4[:, CT + ci : CT + ci + 1],
                    in1=mv[:, 1:2],
                )

        return t

    for ci in range(CT):
        thunks.append(stats_ci(ci))

    state = {}

    def reduce1():
        psg = psG.tile([GPT, 2 * CT], f32, tag="psx", name="psg")
        nc.tensor.matmul(psg, sel_r, stats4, start=True, stop=True)
        state["psg"] = psg
        vpe = gn_tmp.tile([GPT, CT], f32, tag="vpe", name="vpe")
        with nc.allow_low_precision(reason="f32r group mean"):
            nc.vector.tensor_scalar_mul(g2[:, 0:CT], psg[:, 0:CT], 1.0 / GSZ)
        nc.vector.tensor_scalar(
            out=vpe, in0=psg[:, CT : 2 * CT], scalar1=1.0 / GSZ, scalar2=EPS,
            op0=OP.mult, op1=OP.add,
        )
        state["vpe"] = vpe

    thunks.append(reduce1)

    def reduce2():
        vpe = state["vpe"]
        m2 = gn_tmp.tile([GPT, CT], f32, tag="m2", name="m2")
        nc.vector.tensor_tensor(out=m2, in0=g2[:, 0:CT], in1=g2[:, 0:CT], op=OP.mult)
        nc.vector.tensor_sub(vpe, vpe, m2)  # var + eps
        # rstd = rsqrt(var+eps): bit-trick seed + 2 Newton steps (DVE only)
        yr = gn_tmp.tile([GPT, CT], f32, tag="yr", name="yr")
        yi = yr.bitcast(i32)
        nc.vector.tensor_scalar(
            out=yi, in0=vpe.bitcast(i32), scalar1=1, scalar2=None,
            op0=OP.arith_shift_right,
        )
        nc.vector.tensor_tensor(out=yi, in0=magic_sb, in1=yi, op=OP.subtract)
        tn = gn_tmp.tile([GPT, CT], f32, tag="tn", name="tn")
        for _ in range(2):
            nc.vector.tensor_tensor(out=tn, in0=yr, in1=yr, op=OP.mult)
            nc.vector.tensor_tensor(out=tn, in0=tn, in1=vpe, op=OP.mult)
            nc.vector.tensor_scalar(
                out=tn, in0=tn, scalar1=-0.5, scalar2=1.5, op0=OP.mult, op1=OP.add
            )
            nc.vector.tensor_tensor(out=yr, in0=yr, in1=tn, op=OP.mult)
        with nc.allow_low_precision(reason="f32r group rstd"):
            nc.vector.tensor_copy(out=g2[:, CT : 2 * CT], in_=yr)

    thunks.append(reduce2)

    def reduce3():
        psb = psG.tile([P, 2 * CT], f32, tag="psx", name="psb")
        nc.tensor.matmul(psb, selT_r, g2, start=True, stop=True)
        for ci in range(CT):
            nc.vector.tensor_tensor(
                out=AB[:, ci : ci + 1],
                in0=psb[:, CT + ci : CT + ci + 1],
                in1=gs_sb[:, ci : ci + 1],
                op=OP.mult,
            )
            nc.vector.tensor_tensor(
                out=AB[:, CT + ci : CT + ci + 1],
                in0=psb[:, ci : ci + 1],
                in1=AB[:, ci : ci + 1],
                op=OP.mult,
            )
            nc.vector.tensor_sub(
                AB[:, CT + ci : CT + ci + 1],
                gb_sb[:, ci : ci + 1],
                AB[:, CT + ci : CT + ci + 1],
            )

    thunks.append(reduce3)

    def h8_ci(ci):
        def t():
            eng = nc.vector if ci % 2 == 0 else nc.gpsimd
            eng.tensor_scalar(
                out=h8[ci],
                in0=xg[ci],
                scalar1=AB[:, ci : ci + 1],
                scalar2=AB[:, CT + ci : CT + ci + 1],
                op0=OP.mult,
                op1=OP.add,
            )

        return t

    for ci in range(CT):
        thunks.append(h8_ci(ci))
    return thunks


def _build_body(nc, tc, env, rep, thunks):
    par = rep % 2
    xg = env["xg"]
    w8 = env["w8"]
    const = env["const"]
    out_d = env["out_d"]
    bq16_sb = env["bq16_sb"]
    bp_sb = env["bp_sb"]
    ones_col = env["ones_col"]
    ones_row = env["ones_row"]
    expc_sb = env["expc_sb"]
    psA = env["psA"]
    psB = env["psB"]
    psR = env["psR"]
    pt_pool = env["pt_pool"]
    sm_pool = env["sm_pool"]
    h2_pool = env["h2_pool"]
    out_pool = env["out_pool"]
    h8 = env[f"h8_{par}"]

    def sprinkle(n=1):
        for _ in range(min(n, len(thunks))):
            thunks.pop(0)()

    if True:
        # ---------------- q/k/v projections ----------------
        q8 = [const.tile([P, N], f8, tag=f"q8_{ci}", name=f"q8_{ci}") for ci in range(CT)]
        cnt = 0
        for nm, dst8, b16 in (("wq", q8, bq16_sb),):
            for oi in range(CT):
                for nb in range(NBLK):
                    ps = psA.tile([P, 512], f32, tag="ps", name="ps_qk")
                    for ci in range(CT):
                        nc.tensor.matmul(
                            ps,
                            w8[(nm, ci)][:, oi * P : (oi + 1) * P],
                            h8[ci][:, nb * 512 : (nb + 1) * 512],
                            start=(ci == 0),
                            stop=(ci == CT - 1),
                        )
                    dst = dst8[oi][:, nb * 512 : (nb + 1) * 512]
                    if cnt % 4 == 0:
                        nc.scalar.activation(
                            out=dst, in_=ps, func=AF.Identity,
                            bias=b16[:, oi : oi + 1], scale=1.0,
                        )
                    else:
                        nc.vector.tensor_scalar(
                            out=dst, in0=ps, scalar1=b16[:, oi : oi + 1],
                            scalar2=None, op0=OP.add,
                        )
                    cnt += 1
        vT8 = [
            const.tile([P, C], f8, tag=f"vT8_{mi}", name=f"vT8_{mi}")
            for mi in range(MT)
        ]
        for mi in range(MT):
            ps = psA.tile([P, 512], f32, tag="ps", name="ps_v")
            for ci in range(CT):
                nc.tensor.matmul(
                    ps,
                    h8[ci][:, mi * P : (mi + 1) * P],
                    w8[("wv", ci)],
                    start=(ci == 0),
                    stop=(ci == CT - 1),
                )
            # bv is folded into bp_eff on the host (softmax rows sum to 1)
            if cnt % 4 == 0:
                nc.scalar.copy(out=vT8[mi], in_=ps)
            else:
                nc.vector.tensor_copy(out=vT8[mi], in_=ps)
            cnt += 1

        # ---------------- attention, software-pipelined over query blocks ----
        # iteration k: scores(k) in 4 chunks, interleaved with normalizer(k-1),
        # PV(k-1) and output-projection(k-2).
        PT = {}
        csum = {}
        rinv = {}
        Rsb = {}

        def emit_S_chunk(q, c):
            if c == 0:
                PT[q] = [
                    pt_pool.tile([P, 512], f8, tag=f"pt{mi}", name=f"pt{mi}")
                    for mi in range(MT)
                ]
                csum[q] = sm_pool.tile([P, 512], f32r, tag="csum", name="csum")
            for mi in range(4 * c, 4 * c + 4):
                ps_s = psA.tile([P, 512], f32, tag="ps", name="ps_s")
                for ci in range(CT):
                    nc.tensor.matmul(
                        ps_s,
                        h8[ci][:, mi * P : (mi + 1) * P],
                        q8[ci][:, q * 512 : (q + 1) * 512],
                        start=(ci == 0),
                        stop=(ci == CT - 1),
                    )
                pt_dst = PT[q][mi]
                nc.scalar.activation(
                    out=pt_dst, in_=ps_s, func=AF.Exp, bias=expc_sb, scale=SCALE_EFF
                )
                with nc.allow_low_precision(reason="f32r softmax colsum"):
                    if mi == 0:
                        nc.gpsimd.tensor_copy(out=csum[q], in_=pt_dst)
                    else:
                        nc.gpsimd.tensor_tensor(
                            out=csum[q], in0=csum[q], in1=pt_dst, op=OP.add
                        )

        def emit_cs(q):
            # cross-partition reduce of the Pool column-sum tree (x ONES_VAL)
            ps_cs = psR.tile([1, 512], f32, tag="psr", name="ps_cs")
            nc.tensor.matmul(ps_cs, ones_col, csum[q], start=True, stop=True)
            rv = sm_pool.tile([1, 512], f32r, tag="rinv", name="rinv")
            with nc.allow_low_precision(reason="softmax normalizer reciprocal"):
                nc.vector.reciprocal(out=rv, in_=ps_cs)
            rinv[q] = rv

        def emit_R(q):
            ps_R = psR.tile([P, 512], f32, tag="psr", name="ps_R")
            nc.tensor.matmul(ps_R, ones_row, rinv[q], start=True, stop=True)
            Rs = sm_pool.tile([P, 512], f32, tag="Rsb", name="Rsb")
            nc.vector.tensor_copy(out=Rs, in_=ps_R)
            Rsb[q] = Rs

        def emit_PV(q, ci):
            ps_pv = psB.tile([P, 512], f32, tag="ps_pv", name="ps_pv")
            for mi in range(MT):
                nc.tensor.matmul(
                    ps_pv,
                    vT8[mi][:, ci * P : (ci + 1) * P],
                    PT[q][mi],
                    start=(mi == 0),
                    stop=(mi == MT - 1),
                )
            tmp = out_pool.tile([P, 512], f32, tag="tmp", name="tmp")
            nc.vector.tensor_tensor(out=tmp, in0=ps_pv, in1=Rsb[q], op=OP.mult)
            ot = out_pool.tile([P, 512], f32, tag="ot", name="ot")
            nc.vector.scalar_tensor_tensor(
                out=ot, in0=tmp, scalar=bp_sb[:, ci : ci + 1],
                in1=xg[ci][:, q * 512 : (q + 1) * 512], op0=OP.add, op1=OP.add,
            )
            nc.gpsimd.dma_start(
                out=out_d[ci * P : (ci + 1) * P, q * 512 : (q + 1) * 512],
                in_=ot,
            )

        for k in range(NBLK + 1):
            if 1 <= k <= NBLK:
                emit_cs(k - 1)
            if k < NBLK:
                emit_S_chunk(k, 0)
            if 1 <= k <= NBLK:
                emit_R(k - 1)
            sprinkle()
            for c in range(1, 4):
                if k < NBLK:
                    emit_S_chunk(k, c)
                sprinkle()
                if 1 <= k <= NBLK:
                    emit_PV(k - 1, c - 1)
                sprinkle()
            if 1 <= k <= NBLK:
                emit_PV(k - 1, 3)
            sprinkle()
        # drain any leftover GN thunks for the next rep
        sprinkle(len(thunks))


_NC = {}


def _get_nc(reps=1):
    if reps not in _NC:
        _NC[reps] = build(reps)
    return _NC[reps]


def _consts():
    sel = np.zeros((P, GPT), np.float32)
    for rr in range(P):
        sel[rr, rr // GSZ] = 1.0
    selT = sel.T.copy()
    return sel, selT


def _prep_w(w):
    """w [o, c] f32 -> four [128, C] fp8 channel-tile slices of 16*w^T:
    slice ci holds [p, o] = 16 * w[o, 128*ci + p]."""
    wT = np.ascontiguousarray((16.0 * np.asarray(w, np.float32)).T)  # [c, o]
    a = wT.reshape(CT, P, wT.shape[1])
    return [np.ascontiguousarray(a[ci]).astype(E4) for ci in range(CT)]


def make_in_maps(inputs):
    x = np.ascontiguousarray(np.asarray(inputs["x"], dtype=np.float32))
    common = {}
    for nm in ("gn_scale", "gn_bias"):
        common[nm] = np.ascontiguousarray(np.asarray(inputs[nm], dtype=np.float32))
    wp = np.asarray(inputs["wp"], np.float32)
    wv = np.asarray(inputs["wv"], np.float32)
    wk = np.asarray(inputs["wk"], np.float32)
    wq = np.asarray(inputs["wq"], np.float32)
    for nm, w in (("wq", wk.T @ wq), ("wv", wp @ wv)):
        slices = _prep_w(w)
        for ci in range(CT):
            common[f"{nm}8_{ci}"] = slices[ci]
    common["bq16"] = np.ascontiguousarray(
        16.0 * (wk.T @ np.asarray(inputs["bq"], np.float32))
    )
    # bv folds exactly into the output-projection bias: attn(v + bv) = attn(v) + bv
    bv = np.asarray(inputs["bv"], np.float32)
    bp = np.asarray(inputs["bp"], np.float32)
    common["bp_eff"] = np.ascontiguousarray(bp + wp @ bv)
    sel, selT = _consts()
    common["sel"] = sel
    common["selT"] = selT
    return [dict(common, x=x[b]) for b in range(B)]


_EXEC = {}


def _get_exec(nc):
    """Build (once) the sharded jitted callable for the 8-core SPMD program."""
    key = id(nc)
    if key in _EXEC:
        return _EXEC[key]
    import jax
    from jax.sharding import Mesh, NamedSharding, PartitionSpec
    from jax.experimental.shard_map import shard_map
    from concourse.bass2jax import _bass_exec_p, install_neuronx_cc_hook

    install_neuronx_cc_hook()
    in_names, out_names, out_avals = [], [], []
    for alloc in nc.m.functions[0].allocations:
        if not isinstance(alloc, mybir.MemoryLocationSet):
            continue
        name = alloc.memorylocations[0].name
        if alloc.kind == "ExternalInput":
            in_names.append(name)
        elif alloc.kind == "ExternalOutput":
            out_names.append(name)
            out_avals.append(
                jax.core.ShapedArray(tuple(alloc.tensor_shape), mybir.dt.np(alloc.dtype))
            )
    all_names = in_names + out_names

    def _body(*args):
        return tuple(
            _bass_exec_p.bind(
                *args,
                out_avals=tuple(out_avals),
                in_names=tuple(all_names),
                out_names=tuple(out_names),
                lowering_input_output_aliases=(),
                sim_require_finite=True,
                sim_require_nnan=True,
                nc=nc,
            )
        )

    devices = jax.devices()[:B]
    mesh = Mesh(np.asarray(devices), ("core",))
    nsh = NamedSharding(mesh, PartitionSpec("core"))
    nsh_rep = NamedSharding(mesh, PartitionSpec())
    # x and partition_id differ per core; weights/consts are replicated so they
    # transfer once and fan out terminal-side.
    sharded_names = {"x", "partition_id"}
    in_specs = tuple(
        PartitionSpec("core") if nm in sharded_names else PartitionSpec()
        for nm in in_names
    ) + (PartitionSpec("core"),) * len(out_names)
    fn = jax.jit(
        shard_map(
            _body,
            mesh=mesh,
            in_specs=in_specs,
            out_specs=(PartitionSpec("core"),) * len(out_names),
            check_rep=False,
        ),
        keep_unused=True,
    )
    st = {
        "fn": fn,
        "in_names": in_names,
        "out_names": out_names,
        "out_avals": out_avals,
        "nsh": nsh,
        "nsh_rep": nsh_rep,
        "sharded_names": sharded_names,
        "hash": None,
        "dev_args": None,
    }
    _EXEC[key] = st
    return st


def kernel(_retried=False, **inputs):
    import hashlib

    import jax

    nc = _get_nc()
    st = _get_exec(nc)

    hsh = hashlib.md5()
    for nm in ("x", "gn_scale", "gn_bias", "wq", "bq", "wk", "bk", "wv", "bv", "wp", "bp"):
        hsh.update(np.ascontiguousarray(np.asarray(inputs[nm], np.float32)).tobytes())
    digest = hsh.digest()
    if st["hash"] != digest or st["dev_args"] is None:
        in_maps = make_in_maps(inputs)

        def _cv(c, nm):
            if nm in in_maps[c]:
                return np.asarray(in_maps[c][nm])
            for alloc in nc.m.functions[0].allocations:
                if (
                    isinstance(alloc, mybir.MemoryLocationSet)
                    and alloc.memorylocations[0].name == nm
                ):
                    return np.full(
                        tuple(alloc.tensor_shape), c, mybir.dt.np(alloc.dtype)
                    )
            raise KeyError(nm)

        dev_args = []
        for nm in st["in_names"]:
            if nm in st["sharded_names"]:
                a = np.concatenate([_cv(c, nm) for c in range(B)], axis=0)
                dev_args.append(jax.device_put(a, st["nsh"]))
            else:
                dev_args.append(jax.device_put(_cv(0, nm), st["nsh_rep"]))
        for a in st["out_avals"]:
            z = np.zeros((B * a.shape[0], *a.shape[1:]), a.dtype)
            dev_args.append(jax.device_put(z, st["nsh"]))
        st["dev_args"] = dev_args
        st["hash"] = digest

    try:
        r = st["fn"](*st["dev_args"])
        jax.block_until_ready(r)
    except Exception:
        # transient device error (e.g. NRT exec-unit wedge): re-place buffers
        # and retry once after a short backoff
        import time as _time

        _time.sleep(10.0)
        if _retried:
            raise
        st["hash"] = None
        st["dev_args"] = None
        return kernel(_retried=True, **inputs)
    out = np.asarray(r[0]).reshape(B, C, N)
    return out.astype(np.float32)

